# revision 2
# baseline (speedup 1.0000x reference)
"""v7: v3 + tail ACT staging copies + PE tiling-mode bundling.

The tensor engine drains on tiling-mode changes (64-row score matmuls vs
128-row ap matmuls). Issuing kt-pairs as [s0,s1,s0x,s1x] then [ap0,ap1,
ap0x,ap1x] halves the mode switches and gives the 64x128 row tiles
(T0/T8) back-to-back same-mode runs in which to overlap.

v2 (bf16 operands, fused N=1024 exp) plus schedule restructuring to keep the
ACT exp stream (the bottleneck: 16.8M exps/core at ~1 el/cycle/lane @1.2GHz)
saturated:

  * input x-tile DMAs alternate between the sync and gpsimd queues; the five
    weight DMAs go up front on gpsimd while sync starts K's x stream.
  * output projection for query-chunk qc and the projection of Q(qc+2) are
    chopped into small steps interleaved into attention(qc+1)'s kt loop, so
    the PE never runs a long non-score block that would starve ACT.
  * per-qc output staging ([128,512] tiles) copied on DVE (GPSIMD cannot
    read PSUM); hT DMA on the gpsimd queue, cT DMA on sync.
"""

import numpy as np

H = 1024
NH = 16
DK = 64
C = 1024
B = 2
S = 2048
T = B * S
NCORES = 8
NG = 4             # head groups
CPC = H // NG      # 256 cols (4 heads) per core
P = 128
TCH = 512          # matmul moving-dim chunk
NHC = H // P       # 8 contraction chunks for projections
SKT = S // P       # 16 key tiles
SQC = S // TCH     # 4 query chunks

MM_DTYPE = "bfloat16"

_CACHE = {}


def _np_io_dtype(mm_dtype):
    if mm_dtype == "bfloat16":
        import ml_dtypes
        return np.dtype(ml_dtypes.bfloat16)
    return np.dtype(np.float32)


def _build_program(loop_n=None, mm_dtype=MM_DTYPE):
    import contextlib

    import concourse.tile as tile
    from concourse import bacc, mybir
    from concourse.masks import make_identity

    fp32 = mybir.dt.float32
    mdt = getattr(mybir.dt, mm_dtype)
    Act = mybir.ActivationFunctionType

    nc = bacc.Bacc("TRN2", target_bir_lowering=False, debug=False, num_devices=NCORES)

    qT = nc.dram_tensor("qT", [H, S], mdt, kind="ExternalInput").ap()
    kT = nc.dram_tensor("kT", [H, S], mdt, kind="ExternalInput").ap()
    vT = nc.dram_tensor("vT", [H, S], mdt, kind="ExternalInput").ap()
    wq = nc.dram_tensor("wq", [H, CPC], mdt, kind="ExternalInput").ap()
    wk = nc.dram_tensor("wk", [H, CPC], mdt, kind="ExternalInput").ap()
    wv = nc.dram_tensor("wv", [H, CPC], mdt, kind="ExternalInput").ap()
    wo = nc.dram_tensor("wo", [CPC, H], mdt, kind="ExternalInput").ap()
    wc = nc.dram_tensor("wc", [CPC, C], mdt, kind="ExternalInput").ap()
    bq = nc.dram_tensor("bq", [CPC, 1], fp32, kind="ExternalInput").ap()
    bk = nc.dram_tensor("bk", [CPC, 1], fp32, kind="ExternalInput").ap()
    bv = nc.dram_tensor("bv", [1, CPC], fp32, kind="ExternalInput").ap()
    hT = nc.dram_tensor("hT", [H, S], fp32, kind="ExternalOutput").ap()
    cT = nc.dram_tensor("cT", [C, S], fp32, kind="ExternalOutput").ap()

    with tile.TileContext(nc) as tc:
        with (
            tc.tile_pool(name="const", bufs=1) as const,
            tc.tile_pool(name="wqkv", bufs=1) as wpool,
            tc.tile_pool(name="acts", bufs=1) as acts,
            tc.tile_pool(name="xin", bufs=3) as xin,
            tc.tile_pool(name="pt", bufs=6) as ptp,
            tc.tile_pool(name="small", bufs=2) as small,
            tc.tile_pool(name="ostage", bufs=4) as ostage,
            tc.tile_pool(name="ps_sh", bufs=2, space="PSUM") as ps_sh,
            tc.tile_pool(name="ps_s", bufs=2, space="PSUM") as ps_s,
            tc.tile_pool(name="ps_a", bufs=2, space="PSUM") as ps_a,
            tc.For_i(0, loop_n, 1) if loop_n else contextlib.nullcontext(),
        ):
            # ---- constants (big weights on the gpsimd queue; sync starts
            # the K x-stream in parallel) ----
            wq_sb = wpool.tile([P, NHC, 2, P], mdt, tag="wq")
            nc.gpsimd.dma_start(wq_sb[:], wq.rearrange("(a p) (u c) -> p a u c", p=P, c=P))
            wk_sb = wpool.tile([P, NHC, 2, P], mdt, tag="wk")
            nc.gpsimd.dma_start(wk_sb[:], wk.rearrange("(a p) (u c) -> p a u c", p=P, c=P))
            wv_sb = wpool.tile([P, NHC, 2, P], mdt, tag="wv")
            nc.gpsimd.dma_start(wv_sb[:], wv.rearrange("(a p) (u c) -> p a u c", p=P, c=P))
            wo_sb = wpool.tile([P, 2, H], mdt, tag="wo")
            nc.gpsimd.dma_start(wo_sb[:], wo.rearrange("(a p) j -> p a j", p=P))
            wc_sb = wpool.tile([P, 2, C], mdt, tag="wc")
            nc.gpsimd.dma_start(wc_sb[:], wc.rearrange("(a p) j -> p a j", p=P))
            bq_sb = const.tile([P, 2], fp32, tag="bq")
            nc.sync.dma_start(bq_sb[:], bq.rearrange("(u p) o -> p (u o)", p=P))
            bk_sb = const.tile([P, 2], fp32, tag="bk")
            nc.sync.dma_start(bk_sb[:], bk.rearrange("(u p) o -> p (u o)", p=P))
            bv_row = const.tile([1, CPC], fp32, tag="bvr")
            nc.sync.dma_start(bv_row[:], bv[:, :])
            bv_bc = const.tile([P, CPC], fp32, tag="bvb")
            nc.gpsimd.partition_broadcast(bv_bc[:], bv_row[:])
            ident_f = const.tile([P, P], fp32, tag="identf")
            make_identity(nc, ident_f[:])
            ident = const.tile([P, P], mdt, tag="ident")
            nc.scalar.activation(ident[:], ident_f[:], Act.Copy)
            ones_f = const.tile([P, SQC, 1], fp32, tag="onesf")
            nc.vector.memset(ones_f[:], 1.0)

            # ---- persistent activations per 512-token chunk ----
            qTs = [[acts.tile([P, TCH], mdt, tag=f"qTs{u}_{t}", name=f"qT{u}_{t}")
                    for t in range(SQC)] for u in range(2)]
            kTs = [[acts.tile([P, TCH], mdt, tag=f"kTs{u}_{t}", name=f"kT{u}_{t}")
                    for t in range(SQC)] for u in range(2)]
            vTs = [[acts.tile([P, TCH], mdt, tag=f"vTs{u}_{t}", name=f"vT{u}_{t}")
                    for t in range(SQC)] for u in range(2)]
            vh = [[acts.tile([P, 4 * 65], mdt, tag=f"vh{h}_{t}", name=f"vh{h}_{t}")
                   for t in range(SQC)] for h in range(4)]
            mTs = [acts.tile([P, S], mdt, tag=f"mTs{u}", name=f"mT{u}") for u in range(2)]

            for h in range(4):
                for t in range(SQC):
                    nc.scalar.activation(
                        vh[h][t][:].rearrange("p (n c) -> p n c", c=65)[:, :, 64:65],
                        ones_f[:, 0:4, :], Act.Copy)

            HCG = 4
            def proj_steps(src_, w_sb, dsts, bias_sb, th):
                tw = slice(th * TCH, (th + 1) * TCH)
                src3 = src_.rearrange("(a p) t -> p a t", p=P)
                st = {}

                def s_dma(hg):
                    def f():
                        x = xin.tile([P, HCG, TCH], mdt, tag="x")
                        eng = nc.sync if hg == 0 else nc.gpsimd
                        eng.dma_start(x[:], src3[:, hg * HCG:(hg + 1) * HCG, tw])
                        st[f"x{hg}"] = x
                    return f

                def s_mms(u, hg):
                    def f():
                        if f"ps{u}" not in st:
                            st[f"ps{u}"] = ps_sh.tile([P, TCH], fp32, tag="mm",
                                                      name=f"psp{u}")
                        ps, x = st[f"ps{u}"], st[f"x{hg}"]
                        for hi in range(HCG):
                            hc = hg * HCG + hi
                            nc.tensor.matmul(
                                ps[:], lhsT=w_sb[:, hc, u, :], rhs=x[:, hi, :],
                                start=(hc == 0), stop=(hc == NHC - 1))
                    return f

                def s_copy(u):
                    def f():
                        ps = st.pop(f"ps{u}")
                        if bias_sb is not None:
                            nc.vector.tensor_scalar_add(
                                dsts[u][th][:], ps[:], bias_sb[:, u:u + 1])
                        else:
                            nc.vector.tensor_copy(dsts[u][th][:], ps[:])
                    return f

                return [s_dma(0), s_dma(1),
                        s_mms(0, 0), s_mms(0, 1), s_copy(0),
                        s_mms(1, 0), s_mms(1, 1), s_copy(1)]

            def v_natural(th):
                for u in range(2):
                    for i in range(TCH // P):
                        tp = ps_sh.tile([P, TCH], fp32, tag="mm")
                        tpv = tp[:, 0:P // 2].bitcast(mdt)
                        nc.tensor.transpose(tpv, vTs[u][th][:, i * P:(i + 1) * P], ident[:])
                        for hh in range(2):
                            h = 2 * u + hh
                            nc.vector.tensor_tensor(
                                vh[h][th][:, i * 65:i * 65 + 64],
                                tpv[:, hh * 64:(hh + 1) * 64],
                                bv_bc[:, h * 64:(h + 1) * 64],
                                op=mybir.AluOpType.add)

            def out_proj_steps(qc, tail=False):
                # 16 steps: one [128,512] output column block each
                qw2 = slice(qc * TCH, (qc + 1) * TCH)
                steps = []
                for j in range(H // P):
                    for m in range(2):
                        def f(j=j, m=m):
                            w_sb, outT = ((wo_sb, hT), (wc_sb, cT))[m]
                            po = ps_sh.tile([P, TCH], fp32, tag="mm", name="po")
                            for u in range(2):
                                nc.tensor.matmul(
                                    po[:], lhsT=w_sb[:, u, j * P:(j + 1) * P],
                                    rhs=mTs[u][:, qw2], start=(u == 0), stop=(u == 1))
                            ot = ostage.tile([P, TCH], fp32, tag="ot", name="ot")
                            if tail and m == 0:
                                nc.scalar.activation(ot[:], po[:], Act.Copy)
                            else:
                                nc.vector.tensor_copy(ot[:], po[:])
                            if m == 0:
                                nc.gpsimd.dma_start(outT[j * P:(j + 1) * P, qw2], ot[:])
                            else:
                                nc.sync.dma_start(outT[j * P:(j + 1) * P, qw2], ot[:])
                        steps.append(f)
                return steps

            def riffle(a, b):
                # a's first two steps (DMAs) up front, then alternate
                out = list(a[:2])
                ai, bi = 2, 0
                while ai < len(a) or bi < len(b):
                    if bi < len(b):
                        out.append(b[bi]); bi += 1
                    if ai < len(a):
                        out.append(a[ai]); ai += 1
                return out

            def attention(qc, extra=()):
                qw = slice(qc * TCH, (qc + 1) * TCH)
                it = iter(extra)
                for u in range(2):
                    ap0 = ps_a.tile([65, TCH], fp32, tag="attn", name="ap0")
                    ap1 = ps_a.tile([65, TCH], fp32, tag="attn", name="ap1")
                    for kt2 in range(0, SKT, 2):
                        pgp = []
                        for kt in (kt2, kt2 + 1):
                            kth, ki = kt // 4, kt % 4
                            kwi = slice(ki * P, (ki + 1) * P)
                            sg = ps_s.tile([P, 2 * TCH], fp32, tag="s", name="sg")
                            nc.tensor.matmul(sg[:, 0:TCH], lhsT=kTs[u][kth][0:64, kwi],
                                             rhs=qTs[u][qc][0:64, :], start=True, stop=True)
                            nc.tensor.matmul(sg[:, TCH:2 * TCH], lhsT=kTs[u][kth][64:128, kwi],
                                             rhs=qTs[u][qc][64:128, :], start=True, stop=True)
                            pg = ptp.tile([P, 2 * TCH], mdt, tag="p")
                            nc.scalar.activation(pg[:], sg[:], Act.Exp, scale=0.125)
                            pgp.append(pg)
                        for kt, pg in zip((kt2, kt2 + 1), pgp):
                            kth, ki = kt // 4, kt % 4
                            nc.tensor.matmul(ap0[:], lhsT=vh[2 * u][kth][:, ki * 65:(ki + 1) * 65],
                                             rhs=pg[:, 0:TCH], start=(kt == 0), stop=(kt == SKT - 1))
                            nc.tensor.matmul(ap1[:], lhsT=vh[2 * u + 1][kth][:, ki * 65:(ki + 1) * 65],
                                             rhs=pg[:, TCH:2 * TCH], start=(kt == 0), stop=(kt == SKT - 1))
                        for _ in range(2):
                            s = next(it, None)
                            if s is not None:
                                s()
                    for hh, ap in ((0, ap0), (1, ap1)):
                        rec = small.tile([1, TCH], fp32, tag="rec")
                        nc.vector.reciprocal(rec[:], ap[64:65, :])
                        rbc = small.tile([64, TCH], fp32, tag="rbc")
                        nc.gpsimd.partition_broadcast(rbc[:], rec[:])
                        nc.vector.tensor_tensor(
                            mTs[u][hh * 64:(hh + 1) * 64, qw], ap[0:64, :], rbc[:],
                            op=mybir.AluOpType.mult)
                for s in it:
                    s()

            # ---- schedule ----
            for th in range(SQC):
                for s in proj_steps(kT, wk_sb, kTs, bk_sb, th):
                    s()
            for s in proj_steps(qT, wq_sb, qTs, bq_sb, 0):
                s()
            for th in range(SQC):
                for s in proj_steps(vT, wv_sb, vTs, None, th):
                    s()
                v_natural(th)

            for qc in range(SQC):
                extra = []
                if qc + 1 < SQC:
                    extra = proj_steps(qT, wq_sb, qTs, bq_sb, qc + 1)
                if qc > 0:
                    prev = out_proj_steps(qc - 1)
                    extra = riffle(extra, prev) if extra else prev
                attention(qc, extra)
            for s in out_proj_steps(SQC - 1, tail=True):
                s()

    nc.compile()
    return nc


def _get_program():
    if "nc" not in _CACHE:
        _CACHE["nc"] = _build_program()
    return _CACHE["nc"]


def make_in_maps(q, k, v, Wq, bq, Wk, bk, Wv, bv, Wo, bo, Wc, bc, mm_dtype=MM_DTYPE):
    iodt = _np_io_dtype(mm_dtype)
    q = np.asarray(q, np.float32).reshape(T, H)
    k = np.asarray(k, np.float32).reshape(T, H)
    v = np.asarray(v, np.float32).reshape(T, H)
    qTb = [np.ascontiguousarray(q[s * S:(s + 1) * S].T).astype(iodt) for s in range(B)]
    kTb = [np.ascontiguousarray(k[s * S:(s + 1) * S].T).astype(iodt) for s in range(B)]
    vTb = [np.ascontiguousarray(v[s * S:(s + 1) * S].T).astype(iodt) for s in range(B)]
    wqg, wkg, wvg, wog, wcg, bqg, bkg, bvg = [], [], [], [], [], [], [], []
    for g in range(NG):
        cs = slice(g * CPC, (g + 1) * CPC)
        wqg.append(np.ascontiguousarray(np.asarray(Wq, np.float32)[:, cs]).astype(iodt))
        wkg.append(np.ascontiguousarray(np.asarray(Wk, np.float32)[:, cs]).astype(iodt))
        wvg.append(np.ascontiguousarray(np.asarray(Wv, np.float32)[:, cs]).astype(iodt))
        wog.append(np.ascontiguousarray(np.asarray(Wo, np.float32)[cs, :]).astype(iodt))
        wcg.append(np.ascontiguousarray(np.asarray(Wc, np.float32)[cs, :]).astype(iodt))
        bqg.append(np.asarray(bq, np.float32)[cs].reshape(CPC, 1).copy())
        bkg.append(np.asarray(bk, np.float32)[cs].reshape(CPC, 1).copy())
        bvg.append(np.asarray(bv, np.float32)[cs].reshape(1, CPC).copy())

    in_maps = []
    for core in range(NCORES):
        g, s = core % NG, core // NG
        in_maps.append({
            "qT": qTb[s], "kT": kTb[s], "vT": vTb[s],
            "wq": wqg[g], "wk": wkg[g], "wv": wvg[g],
            "wo": wog[g], "wc": wcg[g],
            "bq": bqg[g], "bk": bkg[g], "bv": bvg[g],
        })
    return in_maps


def combine_outputs(results, bo, bc):
    h = np.zeros((B, S, H), np.float32)
    cc = np.zeros((B, S, C), np.float32)
    for s in range(B):
        hT_full = np.zeros((H, S), np.float64)
        cT_full = np.zeros((C, S), np.float64)
        for g in range(NG):
            core = s * NG + g
            hT_full += results[core]["hT"]
            cT_full += results[core]["cT"]
        h[s] = hT_full.T.astype(np.float32) + np.asarray(bo, np.float32)
        cc[s] = cT_full.T.astype(np.float32) + np.asarray(bc, np.float32)
    return (cc, h)


def kernel(q, k, v, Wq, bq, Wk, bk, Wv, bv, Wo, bo, Wc, bc):
    from concourse.bass_utils import run_bass_kernel_spmd

    nc = _get_program()
    in_maps = make_in_maps(q, k, v, Wq, bq, Wk, bk, Wv, bv, Wo, bo, Wc, bc)
    res = run_bass_kernel_spmd(nc, in_maps, core_ids=list(range(NCORES)))
    _CACHE["last_results"] = res
    return combine_outputs(res.results, bo, bc)
